# revision 11
# baseline (speedup 1.0000x reference)
"""CPDBlock (rank-decomposed conv block) Trainium2 kernel.

Reference computation (per image):
  y1 = (sum_r w_head[r]) @ x            # 1x1 conv, 256->256
  y2 = conv_(3,1)(y1, w_body)           # 256->64, pad (1,0) in H
  y3 = conv_(1,3)(y2, w_tail) + b_tail  # 64->256, pad (0,1) in W

Algebraic fusion: head folds into body since both are linear:
  y2[r,h,w] = sum_kh (w_body[:, :, kh] @ w_sum) @ x[:, h+kh-1, w]
so the kernel only runs two conv stages:
  fused:  Wc[kh] = w_body[kh] @ w_head.sum(0)  (3x [64,256], host-side)
  tail:   w_tail as-is, bias folded into the PSUM->SBUF eviction.

Sharding: data-parallel over batch, 16 images / 8 cores = 2 images/core.

Pipeline per core, per image, H in chunks of HC rows, output rows in
pairs of groups (2 x NR = 8 rows, N = NR*112 = 448 per matmul):

  fused  : 12 matmuls/pair [K=128, M=64] -> one PSUM tile; groups g/g+1
           land in PE column-halves 0/1 concurrently (tile_position).
  y2p    : ONE full-width ACT copy PSUM->SBUF per pair into [128, NR, 114]:
           partitions 0:64 = group g ranks, 64:128 = group g+1 ranks,
           cols 1..112 = data, cols 0 and 113 zero pads.  The tail's three
           W-taps are then plain AP column offsets (0/1/2).
  tail   : per mo: 6 matmuls [K=64, M=128]; the g/g+1 row-halves execute
           concurrently in PE row-groups (tile_position (0,0)/(64,0)),
           so effective cost is 3x448 cycles -- the roofline for K=192.
  evict  : PSUM->y3t(fp16) with per-partition bias in one op;
           mo=0 on DVE (tensor_scalar_add), mo=1 on ACT (scalar.add).

Tails run one pair behind fused matmuls so the PE never waits on the
ACT copy.  Input DMAs ride the SP HWDGE ring, output DMAs the ACT ring,
and the next chunk's input is issued at the top of the current chunk, so
in / compute / out all overlap.  All stages in fp16 (x, wf, wt, y2p) --
tail weight loads get FWL (2 fp16/cycle) and stay hidden under matmuls.
"""
import os

import numpy as np

import concourse.bass as bass
import concourse.mybir as mybir
import concourse.tile as tile
from concourse import bacc
from concourse.bass_utils import run_bass_kernel_spmd

F32 = mybir.dt.float32
F32R = mybir.dt.float32r
F16 = mybir.dt.float16

B, CIN, COUT, RANK, H, W = 16, 256, 256, 64, 112, 112
NCORES = 8
BL = B // NCORES          # images per core
KO = CIN // 128           # input-channel k-tiles
MO = COUT // 128          # output-channel m-tiles
HC = 56                   # rows per chunk
NCH = H // HC             # chunks per image
NR = 4                    # output rows per matmul group (N = NR*112 = 448)
NG = HC // NR             # groups per chunk
NPAIR = NG // 2           # group pairs per chunk
NY2P = 4                  # y2p ring depth (28 pairs/iter divisible by 4)

LAST_EXEC_NS = None
LAST_IN_MAPS = None


def _build(reps: int = 1, loop_reps: int = 1, xdt=None, odt=None):
    skip = set(filter(None, os.environ.get("CPD_SKIP", "").split(",")))
    fp16 = os.environ.get("CPD_FP16", "1") == "1"
    if xdt is None:
        xdt = F16 if fp16 else F32R
    if odt is None:
        odt = F16 if fp16 else F32
    nc = bacc.Bacc("TRN2", target_bir_lowering=False, debug=False,
                   num_devices=NCORES)
    x_d = nc.dram_tensor("x", [BL, CIN, H, W], xdt, kind="ExternalInput")
    wf_d = nc.dram_tensor("wf", [128, 3, KO, RANK], xdt, kind="ExternalInput")
    wt_d = nc.dram_tensor("wt", [128, MO, 3, 128], xdt, kind="ExternalInput")
    bias_d = nc.dram_tensor("bias", [128, MO], F32, kind="ExternalInput")
    o_d = nc.dram_tensor("o", [BL, COUT, H, W], odt, kind="ExternalOutput")

    with tile.TileContext(nc) as tc:
        with (
            tc.tile_pool(name="wpool", bufs=1) as wpool,
            tc.tile_pool(name="xpool", bufs=2) as xpool,
            tc.tile_pool(name="ypool", bufs=1) as ypool,
            tc.tile_pool(name="opool", bufs=2) as opool,
            tc.tile_pool(name="psf", bufs=2, space="PSUM") as psf,
            tc.tile_pool(name="pst", bufs=1, space="PSUM") as pst,
        ):
            wf = wpool.tile([128, 3, KO, RANK], xdt)
            wt = wpool.tile([128, MO, 3, 128], xdt)
            bias = wpool.tile([128, MO], F32)
            nc.sync.dma_start(wf[:], wf_d[:])
            nc.sync.dma_start(wt[:], wt_d[:])
            nc.sync.dma_start(bias[:], bias_d[:])

            # Persistent pair-stacked y2 buffers; W-pad columns (0 and 113)
            # are zeroed once and never rewritten.
            y2ps = [ypool.tile([128, NR, 114], xdt, tag=f"y2p{i}",
                               name=f"y2p{i}")
                    for i in range(NY2P)]
            for y2p in y2ps:
                nc.vector.memset(y2p[:, :, 0], 0.0)
                nc.vector.memset(y2p[:, :, 113], 0.0)

            chunk_list = [(b, ch) for b in range(BL) for ch in range(NCH)]
            if reps > 1:
                chunk_list = chunk_list * reps
            xts: dict = {}

            def emit_input(ci):
                b, ch = chunk_list[ci]
                h0 = ch * HC
                xv = x_d.ap()[b].rearrange("(ko p) h w -> p ko h w", p=128)
                xt = xpool.tile([128, KO, HC + 2, W], xdt, name="xt")
                if "indma" in skip:
                    # diag-only: mark written so readers have a producer
                    nc.gpsimd.memset(xt[:, 0, 0:1, :], 0.0)
                    xts[ci] = xt
                    return
                # slot i <-> abs image row h0 + i - 1; edge chunks leave the
                # out-of-image slot unwritten (matmul term skipped instead).
                if ch == 0:
                    lo = 1
                else:
                    # rows h0-1, h0 were already DMA'd into the previous
                    # chunk's tile; copy instead of re-reading HBM.
                    nc.gpsimd.tensor_copy(xt[:, :, 0:2, :],
                                          xts[ci - 1][:, :, HC:HC + 2, :])
                    lo = 2
                hi = HC + 2 if ch < NCH - 1 else HC + 1
                bounds = list(range(lo, hi, 14)) + [hi]
                for s0, s1 in zip(bounds[:-1], bounds[1:]):
                    nc.sync.dma_start(
                        xt[:, :, s0:s1, :],
                        xv[:, :, h0 + s0 - 1:h0 + s1 - 1, :])
                xts[ci] = xt

            def emit_fused(ci, p, pfp):
                b, ch = chunk_list[ci]
                xt = xts[ci]
                for ko in range(KO):
                    for kh in (1, 0, 2):
                        for sub in (0, 1):
                            g = 2 * p + sub
                            r0 = g * NR
                            p0 = 64 * sub
                            out_ap = pfp[p0:p0 + 64, :, :]
                            rhs = xt[:, ko, r0 + kh:r0 + kh + NR, :]
                            if ch == 0 and g == 0 and kh == 0:
                                # output row 0 has no row above
                                out_ap = pfp[p0:p0 + 64, 1:NR, :]
                                rhs = xt[:, ko, 1:NR, :]
                            elif (ch == NCH - 1 and g == NG - 1 and kh == 2):
                                # last row has no row below
                                out_ap = pfp[p0:p0 + 64, 0:NR - 1, :]
                                rhs = xt[:, ko, r0 + 2:r0 + 1 + NR, :]
                            nc.tensor.matmul(
                                out_ap,
                                wf[:, kh, ko, :],
                                rhs,
                                start=(ko == 0 and kh == 1),
                                stop=(ko == KO - 1 and kh == 2),
                                tile_position=(0, p0),
                            )

            def emit_tail(ci, p, y2p, y3t):
                b, ch = chunk_list[ci]
                h0 = ch * HC
                r0 = p * 2 * NR
                ov = o_d.ap()[b].rearrange("(mo p) h w -> p mo h w", p=128)
                for mo in range(MO):
                    pt = pst.tile([128, 2, NR, 112], F32,
                                  padded_shape=[128, 2, NR, 128],
                                  tag=f"pt{mo}", name=f"pt{mo}",
                                  bufs=1 if mo == 0 else 2)
                    if "tail" in skip and "evict" not in skip:
                        nc.vector.memset(pt[:, 0, 0:1, 0:1], 0.0)
                    if "tail" not in skip:
                        for t in range(3):
                            for gh in range(2):
                                rb = 64 * gh
                                nc.tensor.matmul(
                                    pt[:, gh, :, :],
                                    wt[rb:rb + 64, mo, t, :],
                                    y2p[rb:rb + 64, :, t:t + 112],
                                    start=(t == 0), stop=(t == 2),
                                    tile_position=(rb, 0),
                                )
                    if "evict" not in skip:
                        out_ap = y3t[:, mo, r0:r0 + 2 * NR, :].rearrange(
                            "p (g r) w -> p g r w", g=2)
                        if mo == 0:
                            nc.vector.tensor_scalar_add(out_ap, pt[:],
                                                        bias[:, 0:1])
                        else:
                            nc.scalar.add(out_ap, pt[:], bias[:, 1:2])
                # output rows complete through r0 + 8; ship every 16 rows
                # (and the 8-row tail piece) on the ACT HWDGE ring.
                if "outdma" in skip:
                    return
                if "evict" in skip and p == 0:
                    # diag-only: mark y3t written so the out-DMA has a producer
                    nc.vector.memset(y3t[:, :, 0:1, :], 0.0)
                done = r0 + 2 * NR
                if done % 16 == 0:
                    s0 = done - 16
                    nc.scalar.dma_start(ov[:, :, h0 + s0:h0 + done, :],
                                        y3t[:, :, s0:done, :])
                elif done == HC:
                    s0 = done - 8
                    nc.scalar.dma_start(ov[:, :, h0 + s0:h0 + done, :],
                                        y3t[:, :, s0:done, :])

            import contextlib
            loop_cm = (tc.For_i(0, loop_reps, 1) if loop_reps > 1
                       else contextlib.nullcontext())
            with loop_cm:
                emit_input(0)
                it = 0
                pending = None
                for ci in range(len(chunk_list)):
                    y3t = opool.tile([128, MO, HC, W], odt, name="y3t")
                    for p in range(NPAIR):
                        pfp = psf.tile([128, NR, W], F32, name="pfp")
                        if "fused" in skip and "copy" not in skip:
                            nc.vector.memset(pfp[:, 0:1, 0:1], 0.0)
                        if "fused" not in skip:
                            emit_fused(ci, p, pfp)
                        y2p = y2ps[it % NY2P]
                        it += 1
                        if "copy" not in skip:
                            nc.scalar.copy(y2p[:, :, 1:113], pfp[:])
                        if p == 0 and ci + 1 < len(chunk_list):
                            emit_input(ci + 1)
                        if pending is not None:
                            emit_tail(*pending)
                        pending = (ci, p, y2p, y3t)
                    # keep at most the current chunk's xt + next alive
                    if ci - 1 in xts:
                        del xts[ci - 1]
                emit_tail(*pending)
    nc.compile()
    return nc


_NC_CACHE = None


def kernel(x, w_head, w_body, w_tail, b_tail):
    global _NC_CACHE, LAST_EXEC_NS
    x = np.ascontiguousarray(np.asarray(x, dtype=np.float32))
    w_head = np.asarray(w_head, dtype=np.float32)
    w_body = np.asarray(w_body, dtype=np.float32)
    w_tail = np.asarray(w_tail, dtype=np.float32)
    b_tail = np.asarray(b_tail, dtype=np.float32)

    # --- host-side weight prep (tiny) ---
    w_sum = w_head.astype(np.float64).sum(axis=0)          # [COUT, CIN]
    wc = np.einsum("rok,oi->kri", w_body[:, :, :, 0].astype(np.float64),
                   w_sum)                                  # [3, RANK, CIN]
    # wf[p, kh, ko, m] = Wc[kh][m, ko*128+p]
    wf = np.transpose(wc.reshape(3, RANK, KO, 128), (3, 0, 2, 1))
    wf = np.ascontiguousarray(wf.astype(np.float32))

    # wt[p, mo, tap, m] = w_tail[mo*128+m, p%64, 0, tap]  (ranks duplicated
    # in both partition halves so the g/g+1 row-tiles share one tensor)
    wtl = w_tail[:, :, 0, :].reshape(MO, 128, RANK, 3)     # [mo, m, r, kw]
    wt = np.zeros((128, MO, 3, 128), dtype=np.float32)
    for tap in range(3):
        w_t = np.transpose(wtl[:, :, :, tap], (2, 0, 1))   # [r, mo, m]
        wt[0:64, :, tap, :] = w_t
        wt[64:128, :, tap, :] = w_t

    bias = np.ascontiguousarray(b_tail.reshape(MO, 128).T)  # [128, mo]

    fp16 = os.environ.get("CPD_FP16", "1") == "1"
    if fp16:
        x = np.ascontiguousarray(x.astype(np.float16))
        wf = np.ascontiguousarray(wf.astype(np.float16))
        wt = np.ascontiguousarray(wt.astype(np.float16))
    else:
        wt = np.ascontiguousarray(wt)

    if _NC_CACHE is None:
        _NC_CACHE = _build()
    nc = _NC_CACHE

    in_maps = [
        {"x": x[c * BL:(c + 1) * BL], "wf": wf, "wt": wt, "bias": bias}
        for c in range(NCORES)
    ]
    global LAST_IN_MAPS
    LAST_IN_MAPS = in_maps
    trace = os.environ.get("KBENCH_TRACE", "0") == "1"
    res = run_bass_kernel_spmd(nc, in_maps, core_ids=list(range(NCORES)),
                               trace=trace)
    LAST_EXEC_NS = res.exec_time_ns
    out = np.concatenate([r["o"] for r in res.results], axis=0)
    if out.dtype != np.float32:
        out = out.astype(np.float32)
    return out
